# revision 15
# baseline (speedup 1.0000x reference)
"""Trainium2 Bass kernel for nn_Add_PairLinears.

y = sum_a( blockdiag2(W[a]) applied to x[:, perms[a]] ) + sum_a b[a]

Strategy (data-parallel over batch, 8 cores, no collectives):
  - Each core owns a batch shard of 1024 rows.  The host pre-casts x to
    bf16 and pre-transposes the shard to x^T [D, BC] (pure layout prep);
    all data-dependent compute stays on device.
  - On device the kernel is a single streaming pipeline over 16 groups
    of 2 d-tiles: for each group, 7 SWDGE dma_gather calls pull the
    permuted rows (full 2KB rows, idx tables sorted by source row for
    HBM locality; within-tile order is absorbed into lhsT) and one
    HWDGE load streams mixer 0's rows (identity perm -> contiguous).
  - The 2x2 block-diagonal mix exploits PE array packing: the 128x128
    lhsT per (mixer, tile) is block-diagonal, so in 64x64 tiling mode
    each mixer needs only two 64x64 tiles.  Odd mixers' gathered rows
    are placed partition-rotated by 64 so the four tile positions
    (T0/T2/T8/T10) are all used; contributions accumulate into two
    PSUM banks, then DVE adds the banks and the scalar engine fuses
    the bias sum_a b[a] while casting to bf16.
  - No on-device transpose, no DRAM spill: HBM traffic per core is
    64MB of row reads + 8MB y writes (vs 80MB for the spill design).
  - Output is stored transposed (y^T, bf16) and unsharded/transposed/
    upcast on host.
"""

import numpy as np
import ml_dtypes

import concourse.bass as bass
import concourse.bacc as bacc
import concourse.tile as tile
from concourse import library_config, mybir
from concourse.bass_utils import run_bass_kernel_spmd

B, D, A = 8192, 4096, 8
N_CORES = 8
BC = B // N_CORES          # 1024 batch rows per core
NJ = D // 128              # 32 d-tiles of 128
NQ = 4                     # SWDGE queues
DEPTH = 8                  # gather staging pipeline depth (j-tiles in flight)
M0DEPTH = 8                # mixer-0 stream depth (fills the startup ramp)
GIDX = (A - 1) * 128       # fused gather idxs per j-tile (mixers 1..7)

F32 = mybir.dt.float32
BF16 = mybir.dt.bfloat16
I16 = mybir.dt.int16

_GRAPH_CACHE = {}
_LAST_RESULTS = None

HB = BC // 2  # batch half consumed per matmul pass (PSUM bank = 512 f32)


def _build_graph():
    nc = bacc.Bacc(None, num_swdge_queues=NQ)

    xt_ext = nc.declare_dram_parameter("xt", [D, BC], BF16, isOutput=False)
    lhsT_ext = nc.declare_dram_parameter("lhsT", [NJ, 128, A * 64], BF16, isOutput=False)
    idx_ext = nc.declare_dram_parameter("idx", [128, NJ * (GIDX // 16)], I16, isOutput=False)
    bsum_ext = nc.declare_dram_parameter("bsum", [128, NJ], F32, isOutput=False)
    yt_ext = nc.declare_dram_parameter("yt", [D, BC], BF16, isOutput=True)

    qn = [0]

    def next_q():
        q = qn[0]
        qn[0] = (q + 1) % NQ
        return q

    with tile.TileContext(nc) as tc:
        with (
            tc.tile_pool(name="const", bufs=1) as constp,
            tc.tile_pool(name="lhs", bufs=1) as lhsp,
            tc.tile_pool(name="m0", bufs=M0DEPTH) as m0p,
            tc.tile_pool(name="g", bufs=DEPTH) as gp,
            tc.tile_pool(name="y", bufs=4) as yp,
            tc.tile_pool(name="ya", bufs=4) as yap,
            tc.tile_pool(name="ps", bufs=8, space="PSUM") as psp,
        ):
            idx_sb = constp.tile([128, NJ * (GIDX // 16)], I16)
            nc.sync.dma_start(out=idx_sb[:], in_=idx_ext[:])
            bsum_sb = constp.tile([128, NJ], F32)
            nc.sync.dma_start(out=bsum_sb[:], in_=bsum_ext[:])

            nc.gpsimd.load_library(library_config.mlp)

            # full lhsT resident up front (4.2MB) - fills the HBM ramp while
            # the gpsimd library loads, before the gather stream starts
            lhs_all = lhsp.tile([128, NJ, A * 64], BF16)
            nc.sync.dma_start(
                out=lhs_all[:], in_=lhsT_ext[:].rearrange("j t m -> t j m"))

            def load_j(j):
                """One fused SWDGE gather for all 7 permuted mixers of tile j
                (mixer slot on the free axis) + mixer-0 HWDGE stream."""
                gt = gp.tile([128, A - 1, BC], BF16, tag="g")
                c0 = j * (GIDX // 16)
                nc.gpsimd.dma_gather(
                    out_ap=gt[:],
                    in_ap=xt_ext[:],
                    idxs_ap=idx_sb[:, c0:c0 + GIDX // 16],
                    num_idxs=GIDX,
                    num_idxs_reg=GIDX,
                    elem_size=BC,
                    queue_num=next_q(),
                    single_packet=False,
                )
                m0 = m0p.tile([128, BC], BF16, tag="m0")
                nc.sync.dma_start(out=m0[:], in_=xt_ext[j * 128:(j + 1) * 128, :])
                return m0, gt

            def mix_j(j, m0, gt):
                ytile = yp.tile([128, BC], BF16, tag="y")
                pmA0 = psp.tile([128, HB], F32, tag="ps")
                pmA1 = psp.tile([128, HB], F32, tag="ps")
                pmB0 = psp.tile([128, HB], F32, tag="ps")
                pmB1 = psp.tile([128, HB], F32, tag="ps")
                pmA = [pmA0, pmA1]
                pmB = [pmB0, pmB1]

                def rhs_half(a, ph, h):
                    if a == 0:
                        return m0[ph * 64:(ph + 1) * 64, h * HB:(h + 1) * HB]
                    return gt[ph * 64:(ph + 1) * 64, a - 1,
                              h * HB:(h + 1) * HB]

                # 64x64 array packing: per grid (pair of mixers) four
                # tiles T0/T10 (even mixer, natural placement) and T2/T8
                # (odd mixer, rows rotated by 64).  Both batch halves run
                # per grid so each position's lhsT is reused back-to-back.
                for gr in range(A // 2):
                    ae, ao = 2 * gr, 2 * gr + 1
                    st = (gr == 0)
                    sp = (gr == A // 2 - 1)
                    for h in range(2):
                        nc.tensor.matmul(
                            pmA[h][0:64, :],
                            lhs_all[0:64, j, ae * 64:(ae + 1) * 64],
                            rhs_half(ae, 0, h),
                            start=st, stop=sp, tile_position=(0, 0))
                        nc.tensor.matmul(
                            pmB[h][64:128, :],
                            lhs_all[64:128, j, ae * 64:(ae + 1) * 64],
                            rhs_half(ae, 1, h),
                            start=st, stop=sp, tile_position=(64, 64))
                        nc.tensor.matmul(
                            pmA[h][64:128, :],
                            lhs_all[0:64, j, ao * 64:(ao + 1) * 64],
                            rhs_half(ao, 0, h),
                            start=st, stop=sp, tile_position=(0, 64))
                        nc.tensor.matmul(
                            pmB[h][0:64, :],
                            lhs_all[64:128, j, ao * 64:(ao + 1) * 64],
                            rhs_half(ao, 1, h),
                            start=st, stop=sp, tile_position=(64, 0))

                for h in range(2):
                    # both banks can't feed one DVE op (src0/src1 not
                    # both PSUM): ACT folds bias into bank A -> f32
                    # SBUF, DVE adds bank B and casts to bf16
                    yA = yap.tile([128, HB], F32, tag="ya")
                    nc.scalar.activation(
                        yA[:],
                        pmA[h][:],
                        mybir.ActivationFunctionType.Identity,
                        bias=bsum_sb[:, j:j + 1],
                    )
                    nc.vector.tensor_add(
                        ytile[:, h * HB:(h + 1) * HB], pmB[h][:], yA[:])
                nc.scalar.dma_start(
                    out=yt_ext[j * 128:(j + 1) * 128, :], in_=ytile[:])

            for j in range(NJ):
                m0, gt = load_j(j)
                mix_j(j, m0, gt)

    nc.compile()
    return nc


# idx table column base per mixer
A_IDX0 = [a * 256 for a in range(A)]


def _host_tables(W, b, perms):
    """Build the device-side constant tables from W/b/perms.

    Per (mixer a, j-tile, output-half hblk) the 64 gather rows are sorted
    by source row (HBM locality) and placed at partition half
    (hblk + a%2) % 2 - odd mixers rotated by 64 so the four 64x64 PE
    tile positions are all used.  lhsT[j, p, a, :] holds the 64x64
    diagonal block row for the gathered row at partition p.
    """
    lhsT = np.zeros((NJ, 128, A, 64), np.float32)
    idx_vals = np.zeros((A, NJ, 128), np.int64)
    Wf = W.reshape(A, D // 2, 2, 2)
    for a in range(A):
        rot = a % 2
        for j in range(NJ):
            for hblk in range(2):
                pos = 128 * j + 64 * hblk + np.arange(64)
                srcs = perms[a, pos].astype(np.int64)
                order = np.argsort(srcs, kind="stable")
                pos_s = pos[order]
                p0 = 64 * ((hblk + rot) % 2)
                idx_vals[a, j, p0:p0 + 64] = srcs[order]
                n = pos_s // 2
                i = pos_s % 2
                o_l = (pos_s - 128 * j - 64 * hblk) & ~1  # local even output
                q = np.arange(64)
                for oo in range(2):
                    lhsT[j, p0 + q, a, o_l + oo] = Wf[a, n, i, oo]

    lhsT = np.ascontiguousarray(
        lhsT.reshape(NJ, 128, A * 64)).astype(ml_dtypes.bfloat16)

    # idx: per j-tile one fused index block covering mixers 1..A-1
    # (element i = mixer 1 + i//128, partition i%128), placement-ordered
    # source rows wrapped over 16 partitions (index i at [i%16, i//16]),
    # replicated into each Q7 core's 16-partition group
    ncols = (A - 1) * 128 // 16
    idx = np.zeros((128, NJ * ncols), np.int16)
    for j in range(NJ):
        flat = idx_vals[1:, j, :].reshape(-1)          # [(A-1)*128]
        w16 = flat.reshape(ncols, 16).astype(np.int16).T
        idx[:, j * ncols:(j + 1) * ncols] = np.tile(w16, (8, 1))

    bsum = np.ascontiguousarray(
        b.astype(np.float64).sum(axis=0).astype(np.float32).reshape(NJ, 128).T)
    ident = np.eye(128, dtype=np.float32).astype(ml_dtypes.bfloat16)
    return lhsT, idx, bsum, ident


def _host_idx_vals(perms):
    """Placement table (which source row sits at partition p of tile (a,j))."""
    idx_vals = np.zeros((A, NJ, 128), np.int64)
    for a in range(A):
        rot = a % 2
        for j in range(NJ):
            for hblk in range(2):
                pos = 128 * j + 64 * hblk + np.arange(64)
                srcs = np.sort(perms[a, pos].astype(np.int64))
                p0 = 64 * ((hblk + rot) % 2)
                idx_vals[a, j, p0:p0 + 64] = srcs
    return idx_vals


def kernel(x, W, b, perms):
    x = np.asarray(x, dtype=np.float32)
    W = np.asarray(W, dtype=np.float32)
    b = np.asarray(b, dtype=np.float32)
    perms = np.asarray(perms)

    lhsT, idx, bsum, ident = _host_tables(W, b, perms)

    if "nc" not in _GRAPH_CACHE:
        _GRAPH_CACHE["nc"] = _build_graph()
    nc = _GRAPH_CACHE["nc"]

    x_bf = x.astype(ml_dtypes.bfloat16)
    in_maps = []
    for c in range(N_CORES):
        m = {
            "lhsT": lhsT,
            "idx": idx,
            "bsum": bsum,
            "xt": np.ascontiguousarray(x_bf[c * BC:(c + 1) * BC].T),
        }
        in_maps.append(m)

    res = run_bass_kernel_spmd(nc, in_maps, core_ids=list(range(N_CORES)))
    global _LAST_RESULTS
    _LAST_RESULTS = res
    y = np.concatenate(
        [np.asarray(res.results[c]["yt"], dtype=np.float32).T for c in range(N_CORES)],
        axis=0,
    )
    return np.ascontiguousarray(y)


# revision 16
# speedup vs baseline: 1.0126x; 1.0126x over previous
"""Trainium2 Bass kernel for nn_Add_PairLinears.

y = sum_a( blockdiag2(W[a]) applied to x[:, perms[a]] ) + sum_a b[a]

Strategy (data-parallel over batch, 8 cores, no collectives):
  - Each core owns a batch shard of 1024 rows.  The host pre-casts x to
    bf16 and pre-transposes the shard to x^T [D, BC] (pure layout prep);
    all data-dependent compute stays on device.
  - On device the kernel is a single streaming pipeline over 16 groups
    of 2 d-tiles: for each group, 7 SWDGE dma_gather calls pull the
    permuted rows (full 2KB rows, idx tables sorted by source row for
    HBM locality; within-tile order is absorbed into lhsT) and one
    HWDGE load streams mixer 0's rows (identity perm -> contiguous).
  - The 2x2 block-diagonal mix exploits PE array packing: the 128x128
    lhsT per (mixer, tile) is block-diagonal, so in 64x64 tiling mode
    each mixer needs only two 64x64 tiles.  Odd mixers' gathered rows
    are placed partition-rotated by 64 so the four tile positions
    (T0/T2/T8/T10) are all used; contributions accumulate into two
    PSUM banks, then DVE adds the banks and the scalar engine fuses
    the bias sum_a b[a] while casting to bf16.
  - No on-device transpose, no DRAM spill: HBM traffic per core is
    64MB of row reads + 8MB y writes (vs 80MB for the spill design).
  - Output is stored transposed (y^T, bf16) and unsharded/transposed/
    upcast on host.
"""

import numpy as np
import ml_dtypes

import concourse.bass as bass
import concourse.bacc as bacc
import concourse.tile as tile
from concourse import library_config, mybir
from concourse.bass_utils import run_bass_kernel_spmd

B, D, A = 8192, 4096, 8
N_CORES = 8
BC = B // N_CORES          # 1024 batch rows per core
NJ = D // 128              # 32 d-tiles of 128
NQ = 4                     # SWDGE queues
DEPTH = 7                  # gather staging pipeline depth (j-tiles in flight)
M0DEPTH = 10               # mixer-0 stream depth (fills the startup ramp)
GIDX = (A - 1) * 128       # fused gather idxs per j-tile (mixers 1..7)

F32 = mybir.dt.float32
BF16 = mybir.dt.bfloat16
I16 = mybir.dt.int16

_GRAPH_CACHE = {}
_LAST_RESULTS = None

HB = BC // 2  # batch half consumed per matmul pass (PSUM bank = 512 f32)


def _build_graph():
    nc = bacc.Bacc(None, num_swdge_queues=NQ)

    xt_ext = nc.declare_dram_parameter("xt", [D, BC], BF16, isOutput=False)
    lhsT_ext = nc.declare_dram_parameter("lhsT", [NJ, 128, A * 64], BF16, isOutput=False)
    idx_ext = nc.declare_dram_parameter("idx", [128, NJ * (GIDX // 16)], I16, isOutput=False)
    bsum_ext = nc.declare_dram_parameter("bsum", [128, NJ], F32, isOutput=False)
    yt_ext = nc.declare_dram_parameter("yt", [D, BC], BF16, isOutput=True)

    qn = [0]

    def next_q():
        q = qn[0]
        qn[0] = (q + 1) % NQ
        return q

    with tile.TileContext(nc) as tc:
        with (
            tc.tile_pool(name="const", bufs=1) as constp,
            tc.tile_pool(name="lhs", bufs=1) as lhsp,
            tc.tile_pool(name="m0", bufs=M0DEPTH) as m0p,
            tc.tile_pool(name="g", bufs=DEPTH) as gp,
            tc.tile_pool(name="y", bufs=4) as yp,
            tc.tile_pool(name="ya", bufs=4) as yap,
            tc.tile_pool(name="ps", bufs=8, space="PSUM") as psp,
        ):
            idx_sb = constp.tile([128, NJ * (GIDX // 16)], I16)
            nc.sync.dma_start(out=idx_sb[:], in_=idx_ext[:])
            bsum_sb = constp.tile([128, NJ], F32)
            nc.sync.dma_start(out=bsum_sb[:], in_=bsum_ext[:])

            nc.gpsimd.load_library(library_config.mlp)

            # full lhsT resident up front (4.2MB) - fills the HBM ramp while
            # the gpsimd library loads, before the gather stream starts
            lhs_all = lhsp.tile([128, NJ, A * 64], BF16)
            nc.sync.dma_start(
                out=lhs_all[:], in_=lhsT_ext[:].rearrange("j t m -> t j m"))

            def load_j(j):
                """One fused SWDGE gather for all 7 permuted mixers of tile j
                (mixer slot on the free axis) + mixer-0 HWDGE stream."""
                gt = gp.tile([128, A - 1, BC], BF16, tag="g")
                c0 = j * (GIDX // 16)
                nc.gpsimd.dma_gather(
                    out_ap=gt[:],
                    in_ap=xt_ext[:],
                    idxs_ap=idx_sb[:, c0:c0 + GIDX // 16],
                    num_idxs=GIDX,
                    num_idxs_reg=GIDX,
                    elem_size=BC,
                    queue_num=next_q(),
                    single_packet=False,
                )
                m0 = m0p.tile([128, BC], BF16, tag="m0")
                nc.sync.dma_start(out=m0[:], in_=xt_ext[j * 128:(j + 1) * 128, :])
                return m0, gt

            def mix_j(j, m0, gt):
                ytile = yp.tile([128, BC], BF16, tag="y")
                pmA0 = psp.tile([128, HB], F32, tag="ps")
                pmA1 = psp.tile([128, HB], F32, tag="ps")
                pmB0 = psp.tile([128, HB], F32, tag="ps")
                pmB1 = psp.tile([128, HB], F32, tag="ps")
                pmA = [pmA0, pmA1]
                pmB = [pmB0, pmB1]

                def rhs_half(a, ph, h):
                    if a == 0:
                        return m0[ph * 64:(ph + 1) * 64, h * HB:(h + 1) * HB]
                    return gt[ph * 64:(ph + 1) * 64, a - 1,
                              h * HB:(h + 1) * HB]

                # 64x64 array packing: per grid (pair of mixers) four
                # tiles T0/T10 (even mixer, natural placement) and T2/T8
                # (odd mixer, rows rotated by 64).  Both batch halves run
                # per grid so each position's lhsT is reused back-to-back.
                for gr in range(A // 2):
                    ae, ao = 2 * gr, 2 * gr + 1
                    st = (gr == 0)
                    sp = (gr == A // 2 - 1)
                    for h in range(2):
                        nc.tensor.matmul(
                            pmA[h][0:64, :],
                            lhs_all[0:64, j, ae * 64:(ae + 1) * 64],
                            rhs_half(ae, 0, h),
                            start=st, stop=sp, tile_position=(0, 0))
                        nc.tensor.matmul(
                            pmB[h][64:128, :],
                            lhs_all[64:128, j, ae * 64:(ae + 1) * 64],
                            rhs_half(ae, 1, h),
                            start=st, stop=sp, tile_position=(64, 64))
                        nc.tensor.matmul(
                            pmA[h][64:128, :],
                            lhs_all[0:64, j, ao * 64:(ao + 1) * 64],
                            rhs_half(ao, 0, h),
                            start=st, stop=sp, tile_position=(0, 64))
                        nc.tensor.matmul(
                            pmB[h][0:64, :],
                            lhs_all[64:128, j, ao * 64:(ao + 1) * 64],
                            rhs_half(ao, 1, h),
                            start=st, stop=sp, tile_position=(64, 0))

                for h in range(2):
                    # both banks can't feed one DVE op (src0/src1 not
                    # both PSUM): ACT folds bias into bank A -> f32
                    # SBUF, DVE adds bank B and casts to bf16
                    yA = yap.tile([128, HB], F32, tag="ya")
                    nc.scalar.activation(
                        yA[:],
                        pmA[h][:],
                        mybir.ActivationFunctionType.Identity,
                        bias=bsum_sb[:, j:j + 1],
                    )
                    nc.vector.tensor_add(
                        ytile[:, h * HB:(h + 1) * HB], pmB[h][:], yA[:])
                nc.scalar.dma_start(
                    out=yt_ext[j * 128:(j + 1) * 128, :], in_=ytile[:])

            for j in range(NJ):
                m0, gt = load_j(j)
                mix_j(j, m0, gt)

    nc.compile()
    return nc


# idx table column base per mixer
A_IDX0 = [a * 256 for a in range(A)]


def _host_tables(W, b, perms):
    """Build the device-side constant tables from W/b/perms.

    Per (mixer a, j-tile, output-half hblk) the 64 gather rows are sorted
    by source row (HBM locality) and placed at partition half
    (hblk + a%2) % 2 - odd mixers rotated by 64 so the four 64x64 PE
    tile positions are all used.  lhsT[j, p, a, :] holds the 64x64
    diagonal block row for the gathered row at partition p.
    """
    lhsT = np.zeros((NJ, 128, A, 64), np.float32)
    idx_vals = np.zeros((A, NJ, 128), np.int64)
    Wf = W.reshape(A, D // 2, 2, 2)
    for a in range(A):
        rot = a % 2
        for j in range(NJ):
            for hblk in range(2):
                pos = 128 * j + 64 * hblk + np.arange(64)
                srcs = perms[a, pos].astype(np.int64)
                order = np.argsort(srcs, kind="stable")
                pos_s = pos[order]
                p0 = 64 * ((hblk + rot) % 2)
                idx_vals[a, j, p0:p0 + 64] = srcs[order]
                n = pos_s // 2
                i = pos_s % 2
                o_l = (pos_s - 128 * j - 64 * hblk) & ~1  # local even output
                q = np.arange(64)
                for oo in range(2):
                    lhsT[j, p0 + q, a, o_l + oo] = Wf[a, n, i, oo]

    lhsT = np.ascontiguousarray(
        lhsT.reshape(NJ, 128, A * 64)).astype(ml_dtypes.bfloat16)

    # idx: per j-tile one fused index block covering mixers 1..A-1
    # (element i = mixer 1 + i//128, partition i%128), placement-ordered
    # source rows wrapped over 16 partitions (index i at [i%16, i//16]),
    # replicated into each Q7 core's 16-partition group
    ncols = (A - 1) * 128 // 16
    idx = np.zeros((128, NJ * ncols), np.int16)
    for j in range(NJ):
        flat = idx_vals[1:, j, :].reshape(-1)          # [(A-1)*128]
        w16 = flat.reshape(ncols, 16).astype(np.int16).T
        idx[:, j * ncols:(j + 1) * ncols] = np.tile(w16, (8, 1))

    bsum = np.ascontiguousarray(
        b.astype(np.float64).sum(axis=0).astype(np.float32).reshape(NJ, 128).T)
    ident = np.eye(128, dtype=np.float32).astype(ml_dtypes.bfloat16)
    return lhsT, idx, bsum, ident


def _host_idx_vals(perms):
    """Placement table (which source row sits at partition p of tile (a,j))."""
    idx_vals = np.zeros((A, NJ, 128), np.int64)
    for a in range(A):
        rot = a % 2
        for j in range(NJ):
            for hblk in range(2):
                pos = 128 * j + 64 * hblk + np.arange(64)
                srcs = np.sort(perms[a, pos].astype(np.int64))
                p0 = 64 * ((hblk + rot) % 2)
                idx_vals[a, j, p0:p0 + 64] = srcs
    return idx_vals


def kernel(x, W, b, perms):
    x = np.asarray(x, dtype=np.float32)
    W = np.asarray(W, dtype=np.float32)
    b = np.asarray(b, dtype=np.float32)
    perms = np.asarray(perms)

    lhsT, idx, bsum, ident = _host_tables(W, b, perms)

    if "nc" not in _GRAPH_CACHE:
        _GRAPH_CACHE["nc"] = _build_graph()
    nc = _GRAPH_CACHE["nc"]

    x_bf = x.astype(ml_dtypes.bfloat16)
    in_maps = []
    for c in range(N_CORES):
        m = {
            "lhsT": lhsT,
            "idx": idx,
            "bsum": bsum,
            "xt": np.ascontiguousarray(x_bf[c * BC:(c + 1) * BC].T),
        }
        in_maps.append(m)

    res = run_bass_kernel_spmd(nc, in_maps, core_ids=list(range(N_CORES)))
    global _LAST_RESULTS
    _LAST_RESULTS = res
    y = np.concatenate(
        [np.asarray(res.results[c]["yt"], dtype=np.float32).T for c in range(N_CORES)],
        axis=0,
    )
    return np.ascontiguousarray(y)
